# revision 1
# baseline (speedup 1.0000x reference)
"""Bipartite GNN attention kernel for Trainium2, SPMD across 8 NeuronCores.

Math (per reference):
  u = user @ W_u.T + b_u ; v = item @ W_v.T + b_v
  learn_user = softmax((u @ v.T) * UV_adj * scale, axis=1) @ v + u
  learn_item = softmax((v @ u.T) * VU_adj * scale, axis=1) @ u + v

Sharding: core i owns rows [i*1024, (i+1)*1024) of BOTH outputs; no
collectives (the contracted-side projection is replicated).

v2 design:
- Projections run in float32r (full f32 input precision, FP22 matmul) so
  the u/v terms that reach the output directly keep ~1e-4 accuracy.
- The projected feature matrices fT (feature-major, [h,*]) are stored in
  bf16: both directions' 16K-column matrices fit in SBUF at once, so the
  second direction's projection DMA stream prefetches during the first
  direction's attention loop.
- Scores/aggregation/denominator matmuls run in bf16 (same PE rate as
  f32r, FWL halves weight-load cost). Score/softmax epilogue arithmetic
  (mask multiply, exp input, division, +u) stays f32.
- Row-major v chunks come from 2-byte DMA XBAR transposes of fT instead
  of PE transposes (PE and DVE freed; DMA has headroom).
- S^T layout: scores tile is [item-chunk=128 part, user-rows=512 free];
  exp'd tile feeds aggregation directly as the stationary operand and a
  ones-vector matmul accumulates the softmax denominator.
"""

import sys

sys.path.insert(0, "/opt/trn_rl_repo")

import ml_dtypes
import numpy as np

import concourse.bacc as bacc
import concourse.bass as bass
import concourse.mybir as mybir
import concourse.tile as tile
from concourse.bass_utils import run_bass_kernel_spmd

N = 8192          # users == items
H = 512           # hidden
NCORES = 8
RB = N // NCORES  # 1024 rows per core per direction
KH = H // 128     # 4 h-chunks
NB = N // 128     # 64 column chunks
NRB = RB // 512   # 2 r-blocks of 512
SCALE = float(1.0 / np.sqrt(np.float32(H)))

F32 = mybir.dt.float32
F32R = mybir.dt.float32r
BF16 = mybir.dt.bfloat16


def _r(ap):
    return ap.bitcast(F32R)


def build_nc():
    nc = bacc.Bacc("TRN2", target_bir_lowering=False, debug=False)

    userT = nc.declare_dram_parameter("userT", [H, N], F32, isOutput=False)
    itemT = nc.declare_dram_parameter("itemT", [H, N], F32, isOutput=False)
    userT_blk = nc.declare_dram_parameter("userT_blk", [H, RB], F32, isOutput=False)
    itemT_blk = nc.declare_dram_parameter("itemT_blk", [H, RB], F32, isOutput=False)
    maskA = nc.declare_dram_parameter("maskA", [N, RB], BF16, isOutput=False)
    maskB = nc.declare_dram_parameter("maskB", [N, RB], BF16, isOutput=False)
    W_uT = nc.declare_dram_parameter("W_uT", [H, H], F32, isOutput=False)
    W_vT = nc.declare_dram_parameter("W_vT", [H, H], F32, isOutput=False)
    b_u_p = nc.declare_dram_parameter("b_u_p", [128, KH], F32, isOutput=False)
    b_v_p = nc.declare_dram_parameter("b_v_p", [128, KH], F32, isOutput=False)
    ident = nc.declare_dram_parameter("ident", [128, 128], F32, isOutput=False)
    out = nc.declare_dram_parameter("out", [2 * RB, H], F32, isOutput=True)

    with tile.TileContext(nc) as tc:
        with (
            tc.tile_pool(name="bigA", bufs=1) as bigA,
            tc.tile_pool(name="bigB", bufs=1) as bigB,
            tc.tile_pool(name="qtb", bufs=1) as qtbp,
            tc.tile_pool(name="qt32", bufs=1) as qt32p,
            tc.tile_pool(name="wts", bufs=1) as wts,
            tc.tile_pool(name="stream", bufs=12) as stream,
            tc.tile_pool(name="mask", bufs=3) as maskp,
            tc.tile_pool(name="pf", bufs=2) as pfp,
            tc.tile_pool(name="pb", bufs=3) as pbp,
            tc.tile_pool(name="vchunk", bufs=3) as vchp,
            tc.tile_pool(name="outs", bufs=1) as outsp,
            tc.tile_pool(name="small", bufs=1) as small,
            tc.tile_pool(name="ps_s", bufs=2, space="PSUM") as ps_s,      # 2 banks
            tc.tile_pool(name="ps_tr", bufs=1, space="PSUM") as ps_tr,    # 1 bank
            tc.tile_pool(name="ps_agg", bufs=1, space="PSUM") as ps_agg,  # 4 banks
            tc.tile_pool(name="ps_rs", bufs=1, space="PSUM") as ps_rs,    # 1 bank
        ):
            identity = small.tile([128, 128], F32R, tag="ident")
            nc.sync.dma_start(identity[:], ident[:].bitcast(F32R))
            identity_bf = small.tile([128, 128], BF16, tag="identbf")
            nc.vector.tensor_copy(identity_bf[:], identity[:])
            ones_bf = small.tile([128, 1], BF16, tag="ones")
            nc.vector.memset(ones_bf[:], 1.0)
            zbias = small.tile([128, 1], F32, tag="zbias")
            nc.vector.memset(zbias[:], 0.0)
            bu_sb = small.tile([128, KH], F32, tag="bu")
            nc.sync.dma_start(bu_sb[:], b_u_p[:])
            bv_sb = small.tile([128, KH], F32, tag="bv")
            nc.sync.dma_start(bv_sb[:], b_v_p[:])

            def direction(big_pool, pname, featT_dram, qT_blk_dram, w_feat_dram,
                          w_q_dram, bias_feat, bias_q, mask_dram, out_base):
                wq = [wts.tile([128, H], F32R, tag=f"w{k}", name=f"{pname}wq{k}")
                      for k in range(KH)]
                for k in range(KH):
                    for h in range(2):
                        nc.sync.dma_start(
                            wq[k][:, h * 256:(h + 1) * 256],
                            w_q_dram[k * 128:(k + 1) * 128,
                                     h * 256:(h + 1) * 256].bitcast(F32R))

                # -- project q^T block first: bf16 (scores rhs) + f32r --
                qTb = [qtbp.tile([128, RB], BF16, tag=f"qTb{m}", name=f"{pname}qTb{m}")
                       for m in range(KH)]
                qT32 = [qt32p.tile([128, RB], F32R, tag=f"qT32{m}",
                                   name=f"{pname}qT32{m}") for m in range(KH)]
                qt_in = [stream.tile([128, 512], F32R, tag="ft_in",
                                     name=f"{pname}qt{j}_{k}")
                         for j in range(2) for k in range(KH)]
                for j in range(2):
                    for k in range(KH):
                        nc.sync.dma_start(
                            qt_in[j * KH + k][:],
                            qT_blk_dram[k * 128:(k + 1) * 128,
                                        j * 512:(j + 1) * 512].bitcast(F32R))
                for m in range(KH):
                    ps0 = ps_s.tile([128, 512], F32, tag="s")
                    ps1 = ps_s.tile([128, 512], F32, tag="s")
                    for k in range(KH):
                        nc.tensor.matmul(
                            ps0[:], _r(wq[k][:, m * 128:(m + 1) * 128]),
                            qt_in[k][:], start=(k == 0), stop=(k == KH - 1))
                        nc.tensor.matmul(
                            ps1[:], _r(wq[k][:, m * 128:(m + 1) * 128]),
                            qt_in[KH + k][:], start=(k == 0), stop=(k == KH - 1))
                    for j, ps in ((0, ps0), (1, ps1)):
                        nc.vector.tensor_scalar(
                            out=qTb[m][:, j * 512:(j + 1) * 512], in0=ps[:],
                            scalar1=bias_q[:, m:m + 1], scalar2=None,
                            op0=mybir.AluOpType.add)
                        nc.scalar.add(
                            qT32[m][:, j * 512:(j + 1) * 512], ps[:],
                            bias_q[:, m:m + 1])

                wf = [wts.tile([128, H], F32R, tag=f"w{k}", name=f"{pname}wf{k}")
                      for k in range(KH)]
                for k in range(KH):
                    nc.sync.dma_start(
                        wf[k][:], w_feat_dram[k * 128:(k + 1) * 128, :].bitcast(F32R))
                # -- fT projection pairs, emitted interleaved into rb=0 --
                fT = [big_pool.tile([128, N], BF16, tag=f"{pname}fT{m}",
                                    name=f"{pname}fT{m}") for m in range(KH)]

                def emit_pair(np_):
                    ft_in = [stream.tile([128, 512], F32R, tag="ft_in",
                                         name=f"{pname}ft{np_}_{j}_{k}")
                             for j in range(2) for k in range(KH)]
                    for j in range(2):
                        for k in range(KH):
                            nc.sync.dma_start(
                                ft_in[j * KH + k][:],
                                featT_dram[k * 128:(k + 1) * 128,
                                           (2 * np_ + j) * 512:
                                           (2 * np_ + j + 1) * 512].bitcast(F32R))
                    for m in range(KH):
                        ps0 = ps_s.tile([128, 512], F32, tag="s")
                        ps1 = ps_s.tile([128, 512], F32, tag="s")
                        for k in range(KH):
                            nc.tensor.matmul(
                                ps0[:], _r(wf[k][:, m * 128:(m + 1) * 128]),
                                ft_in[k][:], start=(k == 0), stop=(k == KH - 1))
                            nc.tensor.matmul(
                                ps1[:], _r(wf[k][:, m * 128:(m + 1) * 128]),
                                ft_in[KH + k][:], start=(k == 0), stop=(k == KH - 1))
                        nc.vector.tensor_scalar(
                            out=fT[m][:, (2 * np_) * 512:(2 * np_ + 1) * 512],
                            in0=ps0[:], scalar1=bias_feat[:, m:m + 1], scalar2=None,
                            op0=mybir.AluOpType.add)
                        nc.scalar.add(
                            fT[m][:, (2 * np_ + 1) * 512:(2 * np_ + 2) * 512],
                            ps1[:], bias_feat[:, m:m + 1])

                # -- attention main loop (rb=0 carries the projection pairs) --
                for rb in range(NRB):
                    agg = ps_agg.tile([128, KH, 512], F32, tag="agg")
                    rsum4 = ps_rs.tile([128, 4], F32, tag="rs")
                    for b in range(NB):
                        if rb == 0 and b % 8 == 0:
                            emit_pair(b // 8)
                        # row-major feat chunk via PE transpose (bf16),
                        # interleaved with the score matmuls that load the
                        # same fT slice as weights
                        tp = ps_tr.tile([128, 512], BF16, tag="tr")
                        sps = ps_s.tile([128, 512], F32, tag="s")
                        for m in range(KH):
                            nc.tensor.transpose(
                                tp[:, m * 128:(m + 1) * 128],
                                fT[m][:, b * 128:(b + 1) * 128], identity_bf[:])
                            nc.tensor.matmul(
                                sps[:], fT[m][:, b * 128:(b + 1) * 128],
                                qTb[m][:, rb * 512:(rb + 1) * 512],
                                start=(m == 0), stop=(m == KH - 1))
                        v_chunk = vchp.tile([128, 512], BF16, tag="vch")
                        if b % 2 == 0:
                            nc.vector.tensor_copy(v_chunk[:], tp[:])
                        else:
                            nc.scalar.copy(v_chunk[:], tp[:])

                        mt = maskp.tile([128, 512], BF16, tag="mk")
                        nc.sync.dma_start(
                            mt[:], mask_dram[b * 128:(b + 1) * 128,
                                             rb * 512:(rb + 1) * 512])
                        p32 = pfp.tile([128, 512], F32, tag="p32")
                        nc.vector.tensor_tensor(
                            out=p32[:], in0=sps[:], in1=mt[:],
                            op=mybir.AluOpType.mult)
                        pbf = pbp.tile([128, 512], BF16, tag="pbf")
                        nc.scalar.activation(
                            pbf[:], p32[:], mybir.ActivationFunctionType.Exp,
                            bias=zbias[:], scale=SCALE)

                        # aggregation + per-rs denominator (shares lhsT)
                        for rs in range(4):
                            nc.tensor.matmul(
                                agg[:, rs, :], pbf[:, rs * 128:(rs + 1) * 128],
                                v_chunk[:], start=(b == 0), stop=(b == NB - 1))
                            nc.tensor.matmul(
                                rsum4[:, rs:rs + 1], pbf[:, rs * 128:(rs + 1) * 128],
                                ones_bf[:], start=(b == 0), stop=(b == NB - 1))

                    # epilogue: out rows = agg / rsum + q
                    recip = small.tile([128, 4], F32, tag="recip")
                    nc.vector.reciprocal(recip[:], rsum4[:])
                    o_sbs = []
                    for rs in range(4):
                        o_sb = outsp.tile([128, H], F32, tag=f"o{rs}",
                                          name=f"{pname}o{rb}_{rs}")
                        o_sbs.append(o_sb)
                        if rs % 2 == 0:
                            nc.vector.tensor_scalar(
                                out=o_sb[:], in0=agg[:, rs, :],
                                scalar1=recip[:, rs:rs + 1], scalar2=None,
                                op0=mybir.AluOpType.mult)
                        else:
                            nc.scalar.mul(o_sb[:], agg[:, rs, :],
                                          recip[:, rs:rs + 1])
                    for rs in range(4):
                        qp = ps_tr.tile([128, 512], F32R, tag="tr")
                        for m in range(KH):
                            nc.tensor.transpose(
                                qp[:, m * 128:(m + 1) * 128],
                                qT32[m][:, (rb * 4 + rs) * 128:
                                        (rb * 4 + rs + 1) * 128], identity[:])
                        nc.vector.tensor_tensor(
                            out=o_sbs[rs][:], in0=o_sbs[rs][:], in1=qp[:],
                            op=mybir.AluOpType.add)
                        row0 = out_base + rb * 512 + rs * 128
                        nc.sync.dma_start(out[row0:row0 + 128, :], o_sbs[rs][:])

            # UV direction: q = user rows, feat = item, mask^T = VU_adj cols
            direction(bigA, "A", itemT, userT_blk, W_vT, W_uT, bv_sb, bu_sb,
                      maskA, 0)
            # VU direction: q = item rows, feat = user, mask^T = UV_adj cols
            direction(bigB, "B", userT, itemT_blk, W_uT, W_vT, bu_sb, bv_sb,
                      maskB, RB)

    nc.compile()
    return nc


_NC_CACHE = None
TRACE = False
LAST_RESULT = None


def kernel(user, item, UV_adj, VU_adj, W_u, b_u, W_v, b_v):
    global _NC_CACHE, LAST_RESULT
    user = np.asarray(user, dtype=np.float32)
    item = np.asarray(item, dtype=np.float32)
    UV_adj = np.asarray(UV_adj, dtype=np.float32)
    VU_adj = np.asarray(VU_adj, dtype=np.float32)
    W_u = np.asarray(W_u, dtype=np.float32)
    W_v = np.asarray(W_v, dtype=np.float32)
    b_u = np.asarray(b_u, dtype=np.float32)
    b_v = np.asarray(b_v, dtype=np.float32)

    userT = np.ascontiguousarray(user.T)
    itemT = np.ascontiguousarray(item.T)
    W_uT = np.ascontiguousarray(W_u.T)
    W_vT = np.ascontiguousarray(W_v.T)
    b_u_p = np.ascontiguousarray(b_u.reshape(KH, 128).T)
    b_v_p = np.ascontiguousarray(b_v.reshape(KH, 128).T)
    ident = np.eye(128, dtype=np.float32)

    in_maps = []
    for i in range(NCORES):
        sl = slice(i * RB, (i + 1) * RB)
        in_maps.append({
            "userT": userT,
            "itemT": itemT,
            "userT_blk": np.ascontiguousarray(userT[:, sl]),
            "itemT_blk": np.ascontiguousarray(itemT[:, sl]),
            "maskA": np.ascontiguousarray(VU_adj[:, sl].astype(ml_dtypes.bfloat16)),
            "maskB": np.ascontiguousarray(UV_adj[:, sl].astype(ml_dtypes.bfloat16)),
            "W_uT": W_uT,
            "W_vT": W_vT,
            "b_u_p": b_u_p,
            "b_v_p": b_v_p,
            "ident": ident,
        })

    if _NC_CACHE is None:
        _NC_CACHE = build_nc()
    res = run_bass_kernel_spmd(_NC_CACHE, in_maps, core_ids=list(range(NCORES)),
                               trace=TRACE)
    LAST_RESULT = res
    results = res.results
    learn_user = np.concatenate([results[i]["out"][:RB] for i in range(NCORES)], 0)
    learn_item = np.concatenate([results[i]["out"][RB:] for i in range(NCORES)], 0)
    return (learn_user, learn_item)


if __name__ == "__main__":
    nc = build_nc()
    print("built ok")



# revision 6
# speedup vs baseline: 1.7946x; 1.7946x over previous
"""Bipartite GNN attention kernel for Trainium2, SPMD across 8 NeuronCores.

Math (per reference):
  u = user @ W_u.T + b_u ; v = item @ W_v.T + b_v
  learn_user = softmax((u @ v.T) * UV_adj * scale, axis=1) @ v + u
  learn_item = softmax((v @ u.T) * VU_adj * scale, axis=1) @ u + v

Sharding: core i owns rows [i*1024, (i+1)*1024) of BOTH outputs; no
collectives (the contracted-side projection is replicated).

v3 design (fp8 DoubleRow):
- All big matmuls (scores, aggregation, denominator, projections) run in
  fp8e4 with perf_mode=DoubleRow (2 k-chunks per instruction, ~1.5x PE).
- Feature matrices are projected twice: fT [h, N] (feature-major, biased,
  used as score lhsT) and vrow [N, h] (row-major, UNbiased, used as
  aggregation rhs). The missing bias in vrow cancels through softmax:
  P@(v + 1 b^T)/rsum = P@vrow/rsum + b^T, so b_feat is folded into the
  residual qrow instead. This removes all per-block PE transposes.
- Per-core inputs are column-ROLLED so this core's rows are columns
  [0:RB) of both feature matrices; the score rhs (qTb) is then just
  fT_other[:, :, 0:RB] - no separate query projection.
- exp uses bias -ln(32): softmax is shift-invariant, masked entries
  become exactly 1/32 (fp8-exact), max value ~5 stays far below fp8e4
  max 240.
- Residual path stays accurate: qrow = f32r projection of the f32 query
  rows -> bf16, + (b_q + b_feat) broadcast row.
- Aggregation of pair bp-1 is emitted after scores of pair bp so the PE
  never waits on the DVE-mult + Act-exp chain.
"""

import sys

sys.path.insert(0, "/opt/trn_rl_repo")

import ml_dtypes
import numpy as np

import concourse.bacc as bacc
import concourse.bass as bass
import concourse.mybir as mybir
import concourse.tile as tile
from concourse.bass_utils import run_bass_kernel_spmd

N = 8192          # users == items
H = 512           # hidden
NCORES = 8
RB = N // NCORES  # 1024 rows per core per direction
KH = H // 128     # 4 h-chunks
NB = N // 128     # 64 column chunks
NBP = NB // 2     # 32 column-pair chunks (DoubleRow)
NRB = RB // 512   # 2 r-blocks of 512
NJ = N // 512     # 16 512-col blocks for projection streaming
SCALE = float(1.0 / np.sqrt(np.float32(H)))
NLN32 = float(-np.log(32.0))

F32 = mybir.dt.float32
F32R = mybir.dt.float32r
BF16 = mybir.dt.bfloat16
FP8 = mybir.dt.float8e4
NP_FP8 = ml_dtypes.float8_e4m3
DR = mybir.MatmulPerfMode.DoubleRow


def _r(ap):
    return ap.bitcast(F32R)


def build_nc():
    nc = bacc.Bacc("TRN2", target_bir_lowering=False, debug=False)

    featA = nc.declare_dram_parameter("featA", [H, N], FP8, isOutput=False)
    featB = nc.declare_dram_parameter("featB", [H, N], FP8, isOutput=False)
    qtA = nc.declare_dram_parameter("qtA", [H, RB], F32, isOutput=False)
    qtB = nc.declare_dram_parameter("qtB", [H, RB], F32, isOutput=False)
    maskA = nc.declare_dram_parameter("maskA", [N, RB], FP8, isOutput=False)
    maskB = nc.declare_dram_parameter("maskB", [N, RB], FP8, isOutput=False)
    WfA = nc.declare_dram_parameter("WfA", [128, KH, H], FP8, isOutput=False)
    WfB = nc.declare_dram_parameter("WfB", [128, KH, H], FP8, isOutput=False)
    WqA = nc.declare_dram_parameter("WqA", [H, H], F32, isOutput=False)
    WqB = nc.declare_dram_parameter("WqB", [H, H], F32, isOutput=False)
    bfA = nc.declare_dram_parameter("bfA", [128, KH], F32, isOutput=False)
    bfB = nc.declare_dram_parameter("bfB", [128, KH], F32, isOutput=False)
    brow = nc.declare_dram_parameter("brow", [128, H], F32, isOutput=False)
    out = nc.declare_dram_parameter("out", [2 * RB, H], F32, isOutput=True)

    with tile.TileContext(nc) as tc:
        with (
            tc.tile_pool(name="bigA", bufs=1) as bigA,
            tc.tile_pool(name="bigB", bufs=1) as bigB,
            tc.tile_pool(name="wts", bufs=1) as wts,
            tc.tile_pool(name="stream", bufs=6) as stream,
            tc.tile_pool(name="qstream", bufs=4) as qstream,
            tc.tile_pool(name="mask", bufs=3) as maskp,
            tc.tile_pool(name="pf", bufs=3) as pfp,
            tc.tile_pool(name="pb", bufs=3) as pbp,
            tc.tile_pool(name="outs", bufs=1) as outsp,
            tc.tile_pool(name="small", bufs=1) as small,
            tc.tile_pool(name="ps_s", bufs=3, space="PSUM") as ps_s,      # 3 banks
            tc.tile_pool(name="ps_agg", bufs=1, space="PSUM") as ps_agg,  # 4 banks
            tc.tile_pool(name="ps_rs", bufs=1, space="PSUM") as ps_rs,    # 1 bank
        ):
            ones2 = small.tile([128, 2, 16], FP8, tag="ones")
            nc.vector.memset(ones2[:], 1.0)
            nbias = small.tile([128, 1], F32, tag="nbias")
            nc.vector.memset(nbias[:], NLN32)
            brow_sb = small.tile([128, H], F32, tag="brow")
            nc.sync.dma_start(brow_sb[:], brow[:])
            bfA_sb = small.tile([128, KH], F32, tag="bfA")
            nc.sync.dma_start(bfA_sb[:], bfA[:])
            bfB_sb = small.tile([128, KH], F32, tag="bfB")
            nc.sync.dma_start(bfB_sb[:], bfB[:])

            # persistent per-direction tensors
            fT = {}
            vrow = {}
            qrow = {}
            for big_pool, d in ((bigA, "A"), (bigB, "B")):
                fT[d] = big_pool.tile([128, KH, N], FP8, tag=f"fT{d}",
                                      name=f"fT{d}")
                vrow[d] = big_pool.tile([128, NB, H], FP8, tag=f"vrow{d}",
                                        name=f"vrow{d}")
                qrow[d] = big_pool.tile([128, 2 * KH, H], BF16, tag=f"qrow{d}",
                                        name=f"qrow{d}")

            # ---------------- phase 0: projections ----------------
            def project(d, feat_dram, qt_dram, wf_dram, wq_dram, bias_f):
                wfp = wts.tile([128, KH, H], FP8, tag="wfp", name=f"wfp{d}")
                nc.sync.dma_start(wfp[:], wf_dram[:])
                wq = [wts.tile([128, H], F32R, tag=f"wq{k}", name=f"wq{d}{k}")
                      for k in range(KH)]
                for k in range(KH):
                    nc.sync.dma_start(
                        wq[k][:], wq_dram[k * 128:(k + 1) * 128, :].bitcast(F32R))

                # qrow: residual projection, f32r for accuracy
                qt_in = [qstream.tile([128, RB], F32R, tag="qt",
                                      name=f"qt{d}{k}") for k in range(KH)]
                for k in range(KH):
                    nc.sync.dma_start(
                        qt_in[k][:],
                        qt_dram[k * 128:(k + 1) * 128, :].bitcast(F32R))
                for c in range(2 * KH):
                    ps = ps_s.tile([128, H], F32, tag="s")
                    for k in range(KH):
                        nc.tensor.matmul(
                            ps[:], qt_in[k][:, c * 128:(c + 1) * 128], wq[k][:],
                            start=(k == 0), stop=(k == KH - 1))
                    nc.vector.tensor_tensor(
                        out=qrow[d][:, c, :], in0=ps[:], in1=brow_sb[:],
                        op=mybir.AluOpType.add)

                # fT + vrow: fp8 DoubleRow projections
                for j in range(NJ):
                    ft_in = stream.tile([128, KH, 512], FP8, tag="ft",
                                        name=f"ft{d}{j}")
                    for k in range(KH):
                        nc.sync.dma_start(
                            ft_in[:, k, :],
                            feat_dram[k * 128:(k + 1) * 128,
                                      j * 512:(j + 1) * 512])
                    for m in range(KH):
                        ps = ps_s.tile([128, 512], F32, tag="s")
                        for ko in range(2):
                            nc.tensor.matmul(
                                ps[:],
                                wfp[:, 2 * ko:2 * ko + 2, m * 128:(m + 1) * 128],
                                ft_in[:, 2 * ko:2 * ko + 2, :],
                                start=(ko == 0), stop=(ko == 1), perf_mode=DR)
                        if m % 2 == 0:
                            nc.vector.tensor_scalar(
                                out=fT[d][:, m, j * 512:(j + 1) * 512],
                                in0=ps[:], scalar1=bias_f[:, m:m + 1],
                                scalar2=None, op0=mybir.AluOpType.add)
                        else:
                            nc.scalar.add(
                                fT[d][:, m, j * 512:(j + 1) * 512], ps[:],
                                bias_f[:, m:m + 1])
                    for sub in range(4):
                        c = j * 4 + sub
                        ps = ps_s.tile([128, 512], F32, tag="s")
                        for ko in range(2):
                            nc.tensor.matmul(
                                ps[:],
                                ft_in[:, 2 * ko:2 * ko + 2,
                                      sub * 128:(sub + 1) * 128],
                                wfp[:, 2 * ko:2 * ko + 2, :],
                                start=(ko == 0), stop=(ko == 1), perf_mode=DR)
                        if sub % 2 == 0:
                            nc.vector.tensor_copy(vrow[d][:, c, :], ps[:])
                        else:
                            nc.scalar.copy(vrow[d][:, c, :], ps[:])

            project("A", featA, qtA, WfA, WqA, bfA_sb)
            project("B", featB, qtB, WfB, WqB, bfB_sb)

            # ---------------- attention ----------------
            def attention(d, other, mask_dram, out_base):
                myfT = fT[d]
                qTb = fT[other]
                for rb in range(NRB):
                    agg = ps_agg.tile([128, KH, 512], F32, tag="agg")
                    rsum4 = ps_rs.tile([128, 4], F32, tag="rs")
                    pend = None  # (pbf2, bp) waiting for aggregation
                    for bp in range(NBP):
                        sps = []
                        for t in range(2):
                            b = 2 * bp + t
                            sp = ps_s.tile([128, 512], F32, tag="s")
                            for ko in range(2):
                                nc.tensor.matmul(
                                    sp[:],
                                    myfT[:, 2 * ko:2 * ko + 2,
                                         b * 128:(b + 1) * 128],
                                    qTb[:, 2 * ko:2 * ko + 2,
                                        rb * 512:(rb + 1) * 512],
                                    start=(ko == 0), stop=(ko == 1),
                                    perf_mode=DR)
                            sps.append(sp)

                        # aggregate previous pair while DVE/Act chew on this one
                        if pend is not None:
                            emit_agg(*pend)
                        mt = maskp.tile([128, 2, 512], FP8, tag="mk")
                        for t in range(2):
                            nc.sync.dma_start(
                                mt[:, t, :],
                                mask_dram[(2 * bp + t) * 128:
                                          (2 * bp + t + 1) * 128,
                                          rb * 512:(rb + 1) * 512])
                        pbf2 = pbp.tile([128, 2, 512], FP8, tag="pbf")
                        for t in range(2):
                            # gpsimd cannot read PSUM; both mults go on DVE
                            p32 = pfp.tile([128, 512], F32, tag="p32")
                            nc.vector.tensor_tensor(
                                out=p32[:], in0=sps[t][:], in1=mt[:, t, :],
                                op=mybir.AluOpType.mult)
                            nc.scalar.activation(
                                pbf2[:, t, :], p32[:],
                                mybir.ActivationFunctionType.Exp,
                                bias=nbias[:], scale=SCALE)
                        pend = (agg, rsum4, pbf2, bp)
                    emit_agg(*pend)

                    # epilogue: out rows = agg / rsum + qrow
                    recip = small.tile([128, 4], F32, tag="recip")
                    nc.vector.reciprocal(recip[:], rsum4[:])
                    for rs in range(4):
                        o_sb = outsp.tile([128, H], F32, tag=f"o{rs}",
                                          name=f"o{d}{rb}_{rs}")
                        if rs % 2 == 0:
                            nc.vector.tensor_scalar(
                                out=o_sb[:], in0=agg[:, rs, :],
                                scalar1=recip[:, rs:rs + 1], scalar2=None,
                                op0=mybir.AluOpType.mult)
                            nc.vector.tensor_tensor(
                                out=o_sb[:], in0=o_sb[:],
                                in1=qrow[d][:, rb * 4 + rs, :],
                                op=mybir.AluOpType.add)
                        else:
                            nc.scalar.mul(o_sb[:], agg[:, rs, :],
                                          recip[:, rs:rs + 1])
                            nc.gpsimd.tensor_tensor(
                                out=o_sb[:], in0=o_sb[:],
                                in1=qrow[d][:, rb * 4 + rs, :],
                                op=mybir.AluOpType.add)
                        row0 = out_base + rb * 512 + rs * 128
                        nc.sync.dma_start(out[row0:row0 + 128, :], o_sb[:])

            def emit_agg(agg, rsum4, pbf2, bp):
                myvrow = emit_agg.vrow
                for rs in range(4):
                    nc.tensor.matmul(
                        agg[:, rs, :], pbf2[:, :, rs * 128:(rs + 1) * 128],
                        myvrow[:, 2 * bp:2 * bp + 2, :],
                        start=(bp == 0), stop=(bp == NBP - 1), perf_mode=DR)
                    # all 4 columns form ONE psum accumulation group (they
                    # share a 2KB zero region): start only on the very first
                    # matmul, stop only on the very last
                    nc.tensor.matmul(
                        rsum4[:, rs:rs + 1],
                        pbf2[:, :, rs * 128:(rs + 1) * 128],
                        ones2[:, :, 0:1],
                        start=(bp == 0 and rs == 0),
                        stop=(bp == NBP - 1 and rs == 3), perf_mode=DR)

            emit_agg.vrow = vrow["A"]
            attention("A", "B", maskA, 0)
            emit_agg.vrow = vrow["B"]
            attention("B", "A", maskB, RB)

    nc.compile()
    return nc


_NC_CACHE = None
TRACE = False
LAST_RESULT = None


def kernel(user, item, UV_adj, VU_adj, W_u, b_u, W_v, b_v):
    global _NC_CACHE, LAST_RESULT
    user = np.asarray(user, dtype=np.float32)
    item = np.asarray(item, dtype=np.float32)
    UV_adj = np.asarray(UV_adj, dtype=np.float32)
    VU_adj = np.asarray(VU_adj, dtype=np.float32)
    W_u = np.asarray(W_u, dtype=np.float32)
    W_v = np.asarray(W_v, dtype=np.float32)
    b_u = np.asarray(b_u, dtype=np.float32)
    b_v = np.asarray(b_v, dtype=np.float32)

    userT = np.ascontiguousarray(user.T)
    itemT = np.ascontiguousarray(item.T)
    userT8 = userT.astype(NP_FP8)
    itemT8 = itemT.astype(NP_FP8)
    UV8 = UV_adj.astype(NP_FP8)
    VU8 = np.ascontiguousarray(UV8.T)
    W_uT = np.ascontiguousarray(W_u.T)
    W_vT = np.ascontiguousarray(W_v.T)
    # [128, KH, H] fp8 weight layout for DoubleRow projections
    WfA_np = np.ascontiguousarray(
        W_vT.reshape(KH, 128, H).transpose(1, 0, 2).astype(NP_FP8))
    WfB_np = np.ascontiguousarray(
        W_uT.reshape(KH, 128, H).transpose(1, 0, 2).astype(NP_FP8))
    bfA_np = np.ascontiguousarray(b_v.reshape(KH, 128).T)
    bfB_np = np.ascontiguousarray(b_u.reshape(KH, 128).T)
    brow_np = np.ascontiguousarray(
        np.broadcast_to((b_u + b_v)[None, :], (128, H)))

    in_maps = []
    for i in range(NCORES):
        r = i * RB
        sl = slice(r, r + RB)
        in_maps.append({
            # feature matrices with this core's rows rolled to the front
            "featA": np.ascontiguousarray(np.roll(itemT8, -r, axis=1)),
            "featB": np.ascontiguousarray(np.roll(userT8, -r, axis=1)),
            "qtA": np.ascontiguousarray(userT[:, sl]),
            "qtB": np.ascontiguousarray(itemT[:, sl]),
            "maskA": np.ascontiguousarray(np.roll(VU8[:, sl], -r, axis=0)),
            "maskB": np.ascontiguousarray(np.roll(UV8[:, sl], -r, axis=0)),
            "WfA": WfA_np,
            "WfB": WfB_np,
            "WqA": W_uT,
            "WqB": W_vT,
            "bfA": bfA_np,
            "bfB": bfB_np,
            "brow": brow_np,
        })

    if _NC_CACHE is None:
        _NC_CACHE = build_nc()
    res = run_bass_kernel_spmd(_NC_CACHE, in_maps, core_ids=list(range(NCORES)),
                               trace=TRACE)
    LAST_RESULT = res
    results = res.results
    learn_user = np.concatenate([results[i]["out"][:RB] for i in range(NCORES)], 0)
    learn_item = np.concatenate([results[i]["out"][RB:] for i in range(NCORES)], 0)
    return (learn_user, learn_item)


if __name__ == "__main__":
    nc = build_nc()
    print("built ok")
